# revision 55
# baseline (speedup 1.0000x reference)
"""GCN encoder/decoder (gnn_message_passing) Trainium2 kernel.

Pull-model with PE segment-sum aggregation:
  - nodes partitioned across 8 cores (owner-computes on dst); host balances
    nodes so per-(core, src-section, dst-128-window) token counts fit 4
    chunks of 128 (a few designated windows get 5) and per-core totals are
    equal (no conv-boundary skew).
  - the fp16 feature table is split into 4 slot-range sections, each
    distributed with its own slice-AllGather pipelined against the table
    rebuild; gather calls run section-major so next-conv gathers start as
    soon as section 0 lands.
  - per 128-token chunk: dma_gather src rows and segment-sum on the PE:
    psum[feat, seg] += msg[tok, feat]^T @ S[tok, seg], S built on-chip
    (iota==segid).  Partial sums accumulate into zbuf (f32 SBUF) across
    sections; self loops folded in as a dinv^2 pre-scale of zbuf.
  - weight GEMM from fp16 staging after the last section, BN stats fused
    into eviction (accum_out), 1KB AllReduce, scalar affine+ReLU, table
    rebuild (PE transpose + dinv[src] scale + fp16 cast).
"""

import math
import os
import time
from contextlib import ExitStack

import numpy as np

CORES = 8
H = 128
EPS = 1e-5


class Cfg:
    def __init__(self, N, depth=9, sblk=8, cap=8, queues=1, scratch=16384):
        assert N % CORES == 0
        self.N = N
        self.S = N // CORES
        self.NPC = ((self.S + 127) // 128) * 128
        self.NBLK = self.NPC // 128
        self.depth = depth
        self.nconv = 2 * depth + 1
        self.SB = min(sblk, self.NBLK)     # dst blocks per superblock
        self.NSUP = (self.NBLK + self.SB - 1) // self.SB
        self.CAP = cap                     # max chunks per gather call
        self.GSB = 3                       # superblocks per psum group
        self.QUEUES = queues
        self.SCRATCH = scratch
        # table sections: pair-aligned slot ranges, one slice-AllGather each.
        # Packing runs at pair granularity (cap 1024) then splits each pair
        # into two balanced 128-windows.
        self.NPAIR = (self.NBLK + 1) // 2
        q, r = divmod(self.NPAIR, 4)
        secpairs = [q + (1 if i < r else 0) for i in range(4)]
        self.SECWINS = [2 * p for p in secpairs]
        self.SECWINS[3] -= 2 * sum(secpairs) - self.NBLK  # odd NBLK tail
        self.SECSTART = [0]
        for wdt in self.SECWINS:
            self.SECSTART.append(self.SECSTART[-1] + wdt)
        for wdt in self.SECWINS:
            assert 8 * wdt * 128 <= 32767  # gather idx fits int16
        self.SECPAIRS = secpairs
        # overflow pairs (may take 9 chunks = cap 1152), interleaved one per
        # section then a second; reserve pairs are repair-only spill
        ps = [0]
        for p in secpairs:
            ps.append(ps[-1] + p)
        ov1, ov2, ov3 = [], [], []
        self.RESPAIR = []
        for j in range(4):
            lo, span = ps[j], secpairs[j]
            ov1.append(lo + span // 6)
            ov2.append(lo + span // 2)
            ov3.append(lo + (5 * span) // 6)
            self.RESPAIR.append(lo + span // 3)
        self.OVPAIR = ov1 + ov2 + ov3
        self.PSECSTART = ps
        assert cap * 128 <= scratch // 16


FULL = Cfg(100000, queues=4, scratch=49152, cap=8)


# ----------------------------------------------------------------------------
# Host-side preprocessing (sharding / token planning)
# ----------------------------------------------------------------------------

def wrap16(a):
    # token i -> [i % 16, i // 16], replicated to 128 partitions
    b = a.astype(np.int16).reshape(-1, 16).T.copy()
    return np.tile(b, (8, 1))


def balance_nodes(src, dst, N, cfg):
    """Permute nodes -> (core, slot).  Pass 1 equalizes per-core in-edge
    totals; pass A seeds sections (slot ranges) balancing out-degree; pass B
    packs each core's nodes into 256-slot pairs so per-(core, src-section,
    pair) token counts stay <= 1024 (1152 at designated overflow pairs);
    pass C splits each pair into two balanced 128-windows."""
    NPAIR = cfg.NPAIR
    deg_in = np.bincount(dst, minlength=N).astype(np.int64)
    deg_out = np.bincount(src, minlength=N).astype(np.int64)
    sec_of_pair = np.repeat(np.arange(4), cfg.SECPAIRS)

    # ---- pass 1: nodes -> cores (snake over degree-sorted nodes) ----
    order = np.argsort(-deg_in, kind="stable")
    node_core = np.empty(N, np.int8)
    blk = order.reshape(-1, CORES)
    snake = np.arange(CORES)
    for i, row in enumerate(blk):
        node_core[row] = snake if i % 2 == 0 else snake[::-1]

    # ---- pass A: seed sections balancing per-section OUT-degree ----
    node_sec = np.empty(N, np.int8)
    per_core_nodes = []
    for k in range(CORES):
        nodes = np.where(node_core == k)[0]
        per_core_nodes.append(nodes[np.argsort(-deg_in[nodes],
                                               kind="stable")])
        byout = nodes[np.argsort(-deg_out[nodes], kind="stable")]
        seccap = np.array([w * 128 for w in cfg.SECWINS])
        # two LPT iterations: the second starts from the first's final
        # imbalance as a bias, cancelling the slot-capacity tail skew
        bias = np.zeros(4, np.int64)
        for it in range(2):
            fill = np.zeros(4, np.int64)
            tot = bias.copy()
            for n in byout:
                t = np.where(fill < seccap, tot, 1 << 60)
                j = int(t.argmin())
                node_sec[n] = j
                fill[j] += 1
                tot[j] += deg_out[n]
            bias = tot - bias - (tot - bias).mean().astype(np.int64)

    # ---- per-node in-edge profile by src section ----
    prof = np.zeros((N, 4), np.int32)
    np.add.at(prof, (dst, node_sec[src]), 1)

    # ---- pass B: per-core pair packing within sections ----
    node_slot = np.empty(N, np.int32)
    capp = np.full((NPAIR, 4), 1024, np.int64)
    capp[cfg.OVPAIR] = 1152
    capp[cfg.RESPAIR] = 1152
    secmask = np.zeros((4, NPAIR), bool)
    for j in range(4):
        secmask[j, cfg.PSECSTART[j]:cfg.PSECSTART[j + 1]] = True
    pslots = np.array([min(256, cfg.NBLK * 128 - 256 * p)
                       for p in range(NPAIR)])

    for k in range(CORES):
        nodes = per_core_nodes[k]
        v = prof[nodes]                               # [n, 4] degree-sorted
        nsec = node_sec[nodes]
        C = np.zeros((NPAIR, 4), np.int64)
        slots_left = pslots.copy()
        win_of = np.empty(len(nodes), np.int32)
        Tcol = np.maximum(v.sum(axis=0), 1)           # [4]
        weights = np.full((NPAIR, 4), 8.0)
        weights[cfg.OVPAIR] = 9.0
        t_wq = Tcol[None, :] * weights / weights.sum(axis=0)[None, :]
        for i in range(len(nodes)):
            after = C + v[i]
            score = (after / t_wq).max(axis=1) \
                + 1e6 * ((after > capp) & (C <= capp)).sum(axis=1) \
                + np.where(slots_left > 0, 0.0, 1e9) \
                + np.where(secmask[nsec[i]], 0.0, 1e9)
            w = int(score.argmin())
            win_of[i] = w
            C[w] += v[i]
            slots_left[w] -= 1
        # repair: move nodes out of over-cap pairs (same section only)
        stuck = set()
        for _ in range(3000):
            over = np.argwhere(C > capp)
            over = [t for t in over if (t[0], t[1]) not in stuck]
            if not over:
                break
            w1, q = over[0]
            in_w1 = np.where(win_of == w1)[0]
            cand = in_w1[v[in_w1, q] > 0]
            by_q = np.argsort(-v[cand, q], kind="stable")
            cand = np.concatenate([cand[by_q[:8]], cand[by_q[::-1][:8]]])
            sject = sec_of_pair[w1]
            moved = False
            for a in cand:
                room = capp - (C + v[a])              # [NPAIR, 4]
                ok = (room >= 0).all(axis=1) & (slots_left > 0) \
                    & secmask[sject]
                ok[w1] = False
                if ok.any():
                    w2 = int(np.argmax(np.where(ok, room[:, q], -1)))
                    win_of[a] = w2
                    C[w1] -= v[a]
                    C[w2] += v[a]
                    slots_left[w1] += 1
                    slots_left[w2] -= 1
                    moved = True
                    break
                # swap with a low-v[q] node from a same-section pair
                for w2 in np.argsort(np.where(secmask[sject],
                                              C[:, q] - capp[:, q],
                                              1 << 40))[:12]:
                    if w2 == w1 or not secmask[sject][w2]:
                        continue
                    in_w2 = np.where(win_of == w2)[0]
                    if len(in_w2) == 0:
                        continue
                    bs = in_w2[np.argsort(v[in_w2, q], kind="stable")[:4]]
                    for b in bs:
                        if v[b, q] >= v[a, q]:
                            break
                        nC1 = C[w1] - v[a] + v[b]
                        nC2 = C[w2] + v[a] - v[b]
                        if (nC2 > capp[w2]).any() or \
                                (nC1 > capp[w1]).sum() > \
                                (C[w1] > capp[w1]).sum():
                            continue
                        win_of[a], win_of[b] = w2, w1
                        C[w1], C[w2] = nC1, nC2
                        moved = True
                        break
                    if moved:
                        break
                if moved:
                    break
            if not moved:
                stuck.add((int(w1), int(q)))
        # ---- pass C: split each pair into two 128-windows, both under the
        # per-window cap (512, or 576 in overflow pairs) ----
        fill = np.zeros(2 * NPAIR, np.int64)
        for p in range(NPAIR):
            members = np.where(win_of == p)[0]
            if len(members) == 0:
                continue
            mv = v[members]
            o = np.argsort(-mv.sum(axis=1), kind="stable")
            members, mv = members[o], mv[o]
            half = np.zeros((2, 4), np.int64)
            hfill = np.zeros(2, np.int64)
            hcap = np.array([128, min(128, pslots[p] - 128)])
            wcap = capp[p] // 2                       # [4] per-window caps
            tpair = np.maximum(mv.sum(axis=0), 1)
            for i2, n2 in enumerate(members):
                aft = half + mv[i2]                   # [2, 4]
                sc = (aft / tpair).max(axis=1) \
                    + 1e6 * ((aft > wcap) & (half <= wcap)).sum(axis=1) \
                    + np.where(hfill < hcap, 0.0, 1e9)
                h = int(sc.argmin())
                wslot = 2 * p + h
                node_slot[nodes[n2]] = wslot * 128 + fill[wslot]
                fill[wslot] += 1
                half[h] += mv[i2]
                hfill[h] += 1
    return node_core.astype(np.int64), node_slot.astype(np.int64)


def preprocess(x, edge_index, cfg):
    N, S, NPC, NBLK, SB, CAP = (cfg.N, cfg.S, cfg.NPC, cfg.NBLK, cfg.SB,
                                cfg.CAP)
    # self loops are folded in as a dinv^2 pre-scale of zbuf (no tokens)
    src = np.asarray(edge_index[0], np.int64)
    dst = np.asarray(edge_index[1], np.int64)
    deg = np.bincount(dst, minlength=N).astype(np.float32) + 1.0
    dinv = (1.0 / np.sqrt(deg)).astype(np.float32)

    node_core, node_slot = balance_nodes(src, dst, N, cfg)

    NWIN = NBLK
    NG = 4 * NWIN                          # (section, window) groups
    sec_of_win = np.repeat(np.arange(4), cfg.SECWINS)
    secstart = np.asarray(cfg.SECSTART[:4])
    secrows = np.asarray([w * 128 for w in cfg.SECWINS])

    ssec = sec_of_win[node_slot[src] // 128]          # [E] src section
    lrow = (node_core[src] * secrows[ssec]
            + node_slot[src] - secstart[ssec] * 128)  # row in section table
    shard = node_core[dst]

    per_core = []
    cnt = np.zeros((CORES, NG), np.int64)
    for k in range(CORES):
        m = shard == k
        d = node_slot[dst[m]]
        key = ssec[m] * NWIN + d // 128
        order = np.argsort(key, kind="stable")
        per_core.append((lrow[m][order], (d % 128)[order]))
        bounds = np.searchsorted(key[order], np.arange(NG + 1))
        per_core[k] = per_core[k] + (bounds,)
        cnt[k] = np.diff(bounds)

    nch = ((cnt + 127) // 128).max(axis=0)             # [NG] static plan

    # Superblocks are processed in groups of GSB whose psum banks stay
    # resident across all 4 table sections (6 banks + 2 gemm = 8 total);
    # start/stop flags per (group, superblock, bank) span the sections.
    # plan[g][j] -> list of per-superblock nodes (sb, calls).
    GSB = cfg.GSB
    NGRP = (cfg.NSUP + GSB - 1) // GSB
    plan = []
    segcol = 0
    off16 = 0
    for g in range(NGRP):
        sbs = list(range(g * GSB, min((g + 1) * GSB, cfg.NSUP)))
        bank_n = {}
        for sbi in sbs:
            wins = list(range(sbi * SB, min((sbi + 1) * SB, NWIN)))
            for wb in wins:
                bk = (sbi, (wb - sbi * SB) // 4)
                bank_n[bk] = bank_n.get(bk, 0) + sum(
                    int(nch[j * NWIN + wb]) for j in range(4))
        bank_seen = {bk: 0 for bk in bank_n}
        gplan = []
        for j in range(4):
            jnodes = []
            for sbi in sbs:
                wins = list(range(sbi * SB, min((sbi + 1) * SB, NWIN)))
                chunk_ids = [(wb, i) for wb in wins
                             for i in range(int(nch[j * NWIN + wb]))]
                calls = []
                pos = 0
                while pos < len(chunk_ids):
                    take = chunk_ids[pos:pos + CAP]
                    # partial (highest-i) chunks last so their pads become
                    # a trailing run of negative idxs the ucode trims
                    take = sorted(take, key=lambda wi: wi[1])
                    descs = []
                    for jslot, (wb, i) in enumerate(take):
                        bk = (sbi, (wb - sbi * SB) // 4)
                        first = bank_seen[bk] == 0
                        bank_seen[bk] += 1
                        last = bank_seen[bk] == bank_n[bk]
                        descs.append((jslot, wb - sbi * SB, segcol,
                                      bool(first), bool(last)))
                        segcol += 1
                    calls.append(dict(n=len(take), off16=off16, descs=descs,
                                      chunks=take))
                    off16 += len(take) * 8
                    pos += len(take)
                jnodes.append(dict(sb=sbi, calls=calls))
            gplan.append(jnodes)
        plan.append(gplan)
    nchk = segcol

    in_maps = []
    for k in range(CORES):
        lrow_k, seg_k, bounds = per_core[k]
        idx_cols, seg_cols = [], []
        for gplan in plan:
          for j in range(4):
            for node in gplan[j]:
                for call in node["calls"]:
                    L, Sg = [], []
                    for ci, (wb, i) in enumerate(call["chunks"]):
                        gq = j * NWIN + wb
                        lo, hi = int(bounds[gq]), int(bounds[gq + 1])
                        s0 = lo + i * 128
                        last = ci == len(call["chunks"]) - 1
                        rows = np.zeros(128, np.int64)
                        segs = np.full(128, -1, np.int64)
                        n = max(0, min(hi - s0, 128))
                        if n > 0:
                            rows[:n] = lrow_k[s0:s0 + n]
                            segs[:n] = seg_k[s0:s0 + n]
                            if last:
                                rows[n:] = -1  # ucode trims trailing negs
                        L.append(rows)
                        Sg.append(segs)
                    idx_cols.append(wrap16(np.concatenate(L)))
                    seg_cols.append(np.stack(Sg))
        IDX = np.concatenate(idx_cols, axis=1)
        SEGID = np.ascontiguousarray(
            np.concatenate(seg_cols, axis=0).T.astype(np.float16))

        nodes_k = np.where(node_core == k)[0]
        slots_k = node_slot[nodes_k]
        xt = np.zeros((x.shape[1], NPC), dtype=np.float32)
        xt[:, slots_k] = np.asarray(x[nodes_k], np.float32).T
        dv = np.zeros(NPC, dtype=np.float32)
        dv[slots_k] = dinv[nodes_k]
        dinv_nm = np.ascontiguousarray(dv.reshape(NBLK, 128).T)
        dinvb = np.ascontiguousarray(
            np.broadcast_to(dv, (128, NPC)).astype(np.float16))
        in_maps.append({"xT": xt, "gidx": IDX, "segid": SEGID,
                        "dinv_nm": dinv_nm, "dinvb": dinvb})
    return in_maps, plan, nchk, (node_core, node_slot)


# ----------------------------------------------------------------------------
# Device kernel
# ----------------------------------------------------------------------------

def build_nc(cfg, plan, nchk, d_in):
    import concourse.bacc as bacc
    import concourse.bass as bass
    import concourse.mybir as mybir
    import concourse.tile as tile

    f32 = mybir.dt.float32
    f16 = mybir.dt.float16
    i16 = mybir.dt.int16
    AF = mybir.ActivationFunctionType
    ALU = mybir.AluOpType
    AX = mybir.AxisListType

    NPC, NBLK, SB, GSB = cfg.NPC, cfg.NBLK, cfg.SB, cfg.GSB
    depth = cfg.depth
    nconv = cfg.nconv
    MAXSLOT = max(c["n"] for gp in plan for jn in gp for nd in jn
                  for c in nd["calls"])
    NS = (NPC + 511) // 512
    STATC = 2 * cfg.NSUP + 2

    nc = bacc.Bacc("TRN2", target_bir_lowering=False, debug=False,
                   num_devices=CORES,
                   dynamic_dma_scratch_size=cfg.SCRATCH,
                   num_swdge_queues=cfg.QUEUES)

    # ---- I/O ----
    xT_d = nc.dram_tensor("xT", [d_in, NPC], f32, kind="ExternalInput")
    gidx_d = nc.dram_tensor("gidx", [128, nchk * 8], i16, kind="ExternalInput")
    segid_d = nc.dram_tensor("segid", [128, nchk], f16, kind="ExternalInput")
    dinvnm_d = nc.dram_tensor("dinv_nm", [128, NBLK], f32, kind="ExternalInput")
    dinvb_d = nc.dram_tensor("dinvb", [128, NPC], f16, kind="ExternalInput")
    W0_d = nc.dram_tensor("W0", [d_in, H], f32, kind="ExternalInput")
    Ws1_d = nc.dram_tensor("Ws1", [depth, H, H], f16, kind="ExternalInput")
    Ws2_d = nc.dram_tensor("Ws2", [depth - 1, H, H], f16, kind="ExternalInput")
    Wout_d = nc.dram_tensor("Wout", [H, 1], f16, kind="ExternalInput")
    g1_d = nc.dram_tensor("g1T", [H, depth + 1], f32, kind="ExternalInput")
    b1_d = nc.dram_tensor("b1T", [H, depth + 1], f32, kind="ExternalInput")
    g2_d = nc.dram_tensor("g2T", [H, depth - 1], f32, kind="ExternalInput")
    b2_d = nc.dram_tensor("b2T", [H, depth - 1], f32, kind="ExternalInput")
    ident_d = nc.dram_tensor("ident", [128, 128], f32, kind="ExternalInput")
    out_d = nc.dram_tensor("out", [1, NPC], f32, kind="ExternalOutput")

    # ---- internals ----
    tabs = [[nc.dram_tensor(f"tab{i}_{j}",
                            [CORES * cfg.SECWINS[j] * 128, H], f16,
                            addr_space="Shared")
             for j in range(4)] for i in range(2)]
    ulocal = nc.dram_tensor("ulocal", [NPC, H], f16)
    stats_in = nc.dram_tensor("stats_in", [128, 2], f32)
    stats_out = nc.dram_tensor("stats_out", [128, 2], f32, addr_space="Shared")
    xs_d = nc.dram_tensor("xs", [depth, 128, NPC], f32)

    rg = [list(range(CORES))]
    SECROW = [s * 128 for s in cfg.SECSTART]

    with tile.TileContext(nc, num_cores=CORES) as tc, ExitStack() as ctx:
        persist = ctx.enter_context(tc.tile_pool(name="persist", bufs=1))
        msgp = ctx.enter_context(tc.tile_pool(name="msg", bufs=8))
        sp = ctx.enter_context(tc.tile_pool(name="sbld", bufs=5))
        ytp = ctx.enter_context(tc.tile_pool(name="yt", bufs=3))
        stgp = ctx.enter_context(tc.tile_pool(name="stg", bufs=3))
        wp = ctx.enter_context(tc.tile_pool(name="wp", bufs=2))
        skp = ctx.enter_context(tc.tile_pool(name="skp", bufs=3))
        smallp = ctx.enter_context(tc.tile_pool(name="small", bufs=8))
        obp = ctx.enter_context(tc.tile_pool(name="obp", bufs=2))
        accp = ctx.enter_context(tc.tile_pool(name="accp", bufs=1, space="PSUM"))
        pgemm = ctx.enter_context(tc.tile_pool(name="pgemm", bufs=2, space="PSUM"))

        # persistent tiles
        zbuf = persist.tile([128, NPC], f32)
        idx_sb = persist.tile([128, nchk * 8], i16)
        segid_sb = persist.tile([128, nchk], f16)
        dinvb_sb = persist.tile([128, NPC], f16)
        dinvnm_sb = persist.tile([128, NBLK], f32)
        iota_sb = persist.tile([128, 128], f16)
        ident_sb = persist.tile([128, 128], f32)
        sums_sb = persist.tile([128, STATC], f32)
        sumsq_sb = persist.tile([128, STATC], f32)
        stat2_sb = persist.tile([128, 2], f32)
        sqscr = persist.tile([128, 512], f32)
        wout_sb = persist.tile([128, 1], f16)

        # load persistent data (split large loads across DMA queues)
        PIECE = 8192 * 2  # int16 elems per partition-row piece
        tot16 = nchk * 8
        o = 0
        while o < tot16:
            w = min(PIECE, tot16 - o)
            nc.sync.dma_start(out=idx_sb[:, o:o + w], in_=gidx_d[:, o:o + w])
            o += w
        nc.sync.dma_start(out=segid_sb[:], in_=segid_d[:])
        o = 0
        while o < NPC:
            w = min(4096, NPC - o)
            nc.sync.dma_start(out=dinvb_sb[:, o:o + w], in_=dinvb_d[:, o:o + w])
            o += w
        nc.sync.dma_start(out=dinvnm_sb[:], in_=dinvnm_d[:])
        nc.sync.dma_start(out=ident_sb[:], in_=ident_d[:])
        nc.sync.dma_start(out=wout_sb[:], in_=Wout_d[:])
        nc.gpsimd.iota(iota_sb[:], pattern=[[1, 128]], base=0,
                       channel_multiplier=0,
                       allow_small_or_imprecise_dtypes=True)
        # zero-init msg pool buffers: gather pad-trimming leaves slots
        # unwritten, and 0 * NaN garbage would poison the PE segment-sums
        for _ in range(8):
            mz = msgp.tile([128, MAXSLOT, H], f16, tag="msg")
            nc.vector.memset(mz[:], 0.0)

        def gemm_weight(t):
            if t == 1 or t == nconv:
                return None
            w = wp.tile([128, 128], f16, tag="w")
            if t <= depth + 1:
                nc.sync.dma_start(out=w[:], in_=Ws1_d[t - 2])
            else:
                nc.sync.dma_start(out=w[:], in_=Ws2_d[t - depth - 2])
            return w

        def bn_params(t):
            gt = smallp.tile([128, 1], f32, tag="gt")
            bt = smallp.tile([128, 1], f32, tag="bt")
            if t <= depth + 1:
                nc.sync.dma_start(out=gt[:], in_=g1_d[:, t - 1:t])
                nc.sync.dma_start(out=bt[:], in_=b1_d[:, t - 1:t])
            else:
                i = t - depth - 2
                nc.sync.dma_start(out=gt[:], in_=g2_d[:, i:i + 1])
                nc.sync.dma_start(out=bt[:], in_=b2_d[:, i:i + 1])
            return gt, bt

        def build_table(t):
            # zbuf (feature-major fp32) -> transpose -> dinv[src] -> fp16;
            # each section's slice-AllGather fires as soon as its rows are
            # staged so distribution overlaps the rest of the rebuild.
            NB4 = (NBLK + 3) // 4
            agpt = {}
            for j in range(4):
                agpt[(SECROW[j + 1] + 511) // 512 - 1] = j
            for g in range(NB4):
                b0 = 4 * g
                nb = min(4, NBLK - b0)
                st = stgp.tile([128, 4, H], f16, tag="st")
                pt = pgemm.tile([128, 512], f32, tag="pg", name="pt")
                for b_ in range(nb):
                    b = b0 + b_
                    nc.tensor.transpose(
                        pt[:, b_ * 128:(b_ + 1) * 128],
                        zbuf[:, b * 128:(b + 1) * 128], ident_sb[:])
                    nc.vector.tensor_scalar_mul(
                        st[:, b_, :], pt[:, b_ * 128:(b_ + 1) * 128],
                        dinvnm_sb[:, b:b + 1])
                nc.sync.dma_start(
                    out=ulocal[b0 * 128:(b0 + nb) * 128, :]
                    .rearrange("(a p) f -> p a f", p=128),
                    in_=st[:, :nb, :])
                if g in agpt:
                    j = agpt[g]
                    nc.gpsimd.collective_compute(
                        "AllGather", ALU.bypass, replica_groups=rg,
                        ins=[ulocal[SECROW[j]:SECROW[j + 1], :]],
                        outs=[tabs[t % 2][j][:, :]])

        # ---- stage 0: z0.T = W0.T @ xT ----
        w0 = persist.tile([d_in, H], f32)
        nc.sync.dma_start(out=w0[:], in_=W0_d[:])
        for s in range(NS):
            c0 = s * 512
            cw = min(512, NPC - c0)
            xt = skp.tile([d_in, 512], f32, tag="xt")
            nc.sync.dma_start(out=xt[:, :cw], in_=xT_d[:, c0:c0 + cw])
            pg = pgemm.tile([128, 512], f32, tag="pg")
            nc.tensor.matmul(pg[:, :cw], w0[:], xt[:, :cw],
                             start=True, stop=True)
            nc.scalar.copy(zbuf[:, c0:c0 + cw], pg[:, :cw])
        build_table(0)

        # ---- conv layers ----
        qrr = 0
        for t in range(1, nconv + 1):
            w = gemm_weight(t)
            scol = 0
            for gplan in plan:
                # psum banks for this superblock group stay resident across
                # all 4 table sections
                acct = {}
                for node in gplan[0]:
                    sbi = node["sb"]
                    nb_sb = min(SB, NBLK - sbi * SB)
                    for b in range((nb_sb + 3) // 4):
                        acct[(sbi, b)] = accp.tile(
                            [128, 512], f32, tag=f"acct{sbi % GSB}_{b}",
                            name=f"acct{sbi % GSB}_{b}")
                for j in range(4):
                    tabj = tabs[(t - 1) % 2][j]
                    for node in gplan[j]:
                        sbi = node["sb"]
                        for call in node["calls"]:
                            ncall, off16 = call["n"], call["off16"]
                            msg = msgp.tile([128, MAXSLOT, H], f16, tag="msg")
                            nc.gpsimd.dma_gather(
                                msg[:, :ncall, :], tabj[:, :],
                                idx_sb[:, off16:off16 + ncall * 8],
                                ncall * 128, ncall * 128, H,
                                queue_num=qrr % cfg.QUEUES)
                            qrr += 1
                            c0 = call["descs"][0][2]
                            st_ = sp.tile([128, MAXSLOT, 128], f16, tag="S")
                            nc.vector.tensor_tensor(
                                st_[:, :ncall, :],
                                iota_sb[:].unsqueeze(1)
                                .broadcast_to([128, ncall, 128]),
                                segid_sb[:, c0:c0 + ncall].unsqueeze(2)
                                .broadcast_to([128, ncall, 128]),
                                op=ALU.is_equal)
                            for (jslot, jp, segc, first, last) in call["descs"]:
                                nc.tensor.matmul(
                                    acct[(sbi, jp // 4)][:, (jp % 4) * 128:
                                                         (jp % 4) * 128 + 128],
                                    msg[:, jslot, :], st_[:, jslot, :],
                                    start=first, stop=last)

                # ---- evict group (y*dinv + self loop dinv^2*z_prev) ----
                for node in gplan[0]:
                    sbi = node["sb"]
                    nb_sb = min(SB, NBLK - sbi * SB)
                    nb0 = sbi * SB * 128
                    accs = [acct[(sbi, jb // 4)][:, (jb % 4) * 128:
                                                 (jb % 4 + 1) * 128]
                            for jb in range(nb_sb)]
                    if t == 1:
                        for jb in range(nb_sb):
                            cols = slice(nb0 + jb * 128, nb0 + (jb + 1) * 128)
                            stmp = ytp.tile([128, 128], f16, tag="slf",
                                            name="stmp")
                            nc.vector.tensor_mul(stmp[:], zbuf[:, cols],
                                                 dinvb_sb[:, cols])
                            nc.vector.tensor_mul(stmp[:], stmp[:],
                                                 dinvb_sb[:, cols])
                            nc.vector.tensor_mul(zbuf[:, cols], accs[jb],
                                                 dinvb_sb[:, cols])
                            nc.vector.tensor_add(zbuf[:, cols], zbuf[:, cols],
                                                 stmp[:])
                        continue
                    ytmp = ytp.tile([128, SB * 128], f16, tag="ytmp")
                    for jb in range(nb_sb):
                        cols = slice(nb0 + jb * 128, nb0 + (jb + 1) * 128)
                        ycols = slice(jb * 128, (jb + 1) * 128)
                        stmp = ytp.tile([128, 128], f16, tag="slf",
                                        name="stmp")
                        nc.vector.tensor_mul(stmp[:], zbuf[:, cols],
                                             dinvb_sb[:, cols])
                        nc.vector.tensor_mul(stmp[:], stmp[:],
                                             dinvb_sb[:, cols])
                        nc.vector.tensor_mul(ytmp[:, ycols], accs[jb],
                                             dinvb_sb[:, cols])
                        nc.vector.tensor_add(ytmp[:, ycols], ytmp[:, ycols],
                                             stmp[:])
                    for hw_ in range(0, nb_sb * 128, 512):
                        cw = min(512, nb_sb * 128 - hw_)
                        cols = slice(nb0 + hw_, nb0 + hw_ + cw)
                        if t < nconv:
                            pg = pgemm.tile([128, 512], f32, tag="pg")
                            nc.tensor.matmul(pg[:, :cw], w[:],
                                             ytmp[:, hw_:hw_ + cw],
                                             start=True, stop=True)
                            nc.scalar.activation(
                                zbuf[:, cols], pg[:, :cw],
                                AF.Copy, accum_out=sums_sb[:, scol:scol + 1])
                            nc.scalar.activation(
                                sqscr[:, :cw], pg[:, :cw],
                                AF.Square,
                                accum_out=sumsq_sb[:, scol:scol + 1])
                            scol += 1
                        else:
                            po = pgemm.tile([128, 512], f32, tag="pg",
                                            name="po")
                            nc.tensor.matmul(po[0:1, :cw], wout_sb[:],
                                             ytmp[:, hw_:hw_ + cw],
                                             start=True, stop=True)
                            ob = obp.tile([1, 512], f32, tag="ob")
                            nc.scalar.activation(ob[:, :cw], po[0:1, :cw],
                                                 AF.Sigmoid)
                            nc.sync.dma_start(
                                out=out_d[:, nb0 + hw_: nb0 + hw_ + cw],
                                in_=ob[:, :cw])

            if t == nconv:
                break

            # ---- BN stats ----
            if t == 1:
                for s in range(NS):
                    c0 = s * 512
                    cw = min(512, NPC - c0)
                    zsl = zbuf[:, c0:c0 + cw]
                    nc.vector.tensor_reduce(sums_sb[:, s:s + 1], zsl,
                                            axis=AX.X, op=ALU.add)
                    nc.vector.tensor_mul(sqscr[:, :cw], zsl, zsl)
                    nc.vector.tensor_reduce(sumsq_sb[:, s:s + 1], sqscr[:, :cw],
                                            axis=AX.X, op=ALU.add)
                scol = NS
            nc.vector.tensor_reduce(stat2_sb[:, 0:1], sums_sb[:, :scol],
                                    axis=AX.X, op=ALU.add)
            nc.vector.tensor_reduce(stat2_sb[:, 1:2], sumsq_sb[:, :scol],
                                    axis=AX.X, op=ALU.add)
            nc.sync.dma_start(out=stats_in[:, :], in_=stat2_sb[:])
            nc.gpsimd.collective_compute(
                "AllReduce", ALU.add, replica_groups=rg,
                ins=[stats_in[:, :]], outs=[stats_out[:, :]])
            gst = smallp.tile([128, 2], f32, tag="gst")
            nc.sync.dma_start(out=gst[:], in_=stats_out[:, :])

            mean = smallp.tile([128, 1], f32, tag="mean")
            m2 = smallp.tile([128, 1], f32, tag="m2")
            var = smallp.tile([128, 1], f32, tag="var")
            scl = smallp.tile([128, 1], f32, tag="scl")
            sft = smallp.tile([128, 1], f32, tag="sft")
            inv_n = 1.0 / float(cfg.N)
            nc.vector.tensor_scalar_mul(mean[:], gst[:, 0:1], inv_n)
            nc.vector.tensor_scalar_mul(var[:], gst[:, 1:2], inv_n)
            nc.vector.tensor_mul(m2[:], mean[:], mean[:])
            nc.vector.scalar_tensor_tensor(
                var[:], m2[:], -1.0, var[:], op0=ALU.mult, op1=ALU.add)
            nc.vector.tensor_scalar_add(var[:], var[:], EPS)
            gt, bt = bn_params(t)
            nc.scalar.sqrt(scl[:], var[:])
            nc.vector.reciprocal(scl[:], scl[:])
            nc.vector.tensor_mul(scl[:], scl[:], gt[:])
            nc.vector.tensor_mul(sft[:], mean[:], scl[:])
            nc.vector.scalar_tensor_tensor(
                sft[:], sft[:], -1.0, bt[:], op0=ALU.mult, op1=ALU.add)

            # ---- normalize + relu (in place on zbuf) ----
            for s in range(NS):
                c0 = s * 512
                cw = min(512, NPC - c0)
                nc.scalar.activation(zbuf[:, c0:c0 + cw], zbuf[:, c0:c0 + cw],
                                     AF.Relu, bias=sft[:], scale=scl[:])

            # ---- skip add (must precede the table build) ----
            if t + 1 >= depth + 2:
                jj = 2 * depth - t
                for s in range(NS):
                    c0 = s * 512
                    cw = min(512, NPC - c0)
                    sk = skp.tile([128, 512], f32, tag="sk")
                    nc.sync.dma_start(out=sk[:, :cw],
                                      in_=xs_d[jj][:, c0:c0 + cw])
                    nc.vector.tensor_add(zbuf[:, c0:c0 + cw],
                                         zbuf[:, c0:c0 + cw], sk[:, :cw])

            build_table(t)

            # ---- stash xs (reads zbuf only; off the rebuild path) ----
            if t <= depth:
                for s in range(NS):
                    c0 = s * 512
                    cw = min(512, NPC - c0)
                    nc.sync.dma_start(out=xs_d[t - 1][:, c0:c0 + cw],
                                      in_=zbuf[:, c0:c0 + cw])

    nc.compile()
    return nc


# ----------------------------------------------------------------------------
# Entry point
# ----------------------------------------------------------------------------

LAST_INFO = {}


def _run(inputs, cfg):
    from concourse.bass_utils import run_bass_kernel_spmd

    x = np.asarray(inputs["x"], np.float32)
    d_in = x.shape[1]
    t0 = time.time()
    shard_maps, plan, nchk, (node_core, node_slot) = preprocess(
        x, inputs["edge_index"], cfg)
    t1 = time.time()
    nc = build_nc(cfg, plan, nchk, d_in)
    t2 = time.time()

    common = {
        "W0": np.asarray(inputs["W0"], np.float32),
        "Ws1": np.asarray(inputs["Ws1"], np.float16),
        "Ws2": np.asarray(inputs["Ws2"], np.float16),
        "Wout": np.asarray(inputs["Wout"], np.float16),
        "g1T": np.ascontiguousarray(np.asarray(inputs["g1"], np.float32).T),
        "b1T": np.ascontiguousarray(np.asarray(inputs["b1"], np.float32).T),
        "g2T": np.ascontiguousarray(np.asarray(inputs["g2"], np.float32).T),
        "b2T": np.ascontiguousarray(np.asarray(inputs["b2"], np.float32).T),
        "ident": np.eye(128, dtype=np.float32),
    }
    in_maps = [dict(m, **common) for m in shard_maps]
    kw = {}
    if os.environ.get("KBENCH_TRACE"):
        kw = dict(trace=True, tmpdir=os.environ.get("KBENCH_TMPDIR") or None)
    res = run_bass_kernel_spmd(nc, in_maps, list(range(CORES)), **kw)
    t3 = time.time()
    LAST_INFO.update(preprocess_s=t1 - t0, build_s=t2 - t1, run_s=t3 - t2,
                     exec_time_ns=res.exec_time_ns, nchk=nchk)
    per_core = [res.results[k]["out"][0] for k in range(CORES)]
    out = np.empty(cfg.N, np.float32)
    for k in range(CORES):
        m = node_core == k
        out[m] = per_core[k][node_slot[m]]
    return out.reshape(cfg.N, 1).astype(np.float32)


def kernel(**inputs):
    return _run(inputs, FULL)


# revision 56
# speedup vs baseline: 1.1435x; 1.1435x over previous
"""GCN encoder/decoder (gnn_message_passing) Trainium2 kernel.

Pull-model with PE segment-sum aggregation:
  - nodes partitioned across 8 cores (owner-computes on dst); host balances
    nodes so per-(core, src-section, dst-128-window) token counts fit 4
    chunks of 128 (a few designated windows get 5) and per-core totals are
    equal (no conv-boundary skew).
  - the fp16 feature table is split into 4 slot-range sections, each
    distributed with its own slice-AllGather pipelined against the table
    rebuild; gather calls run section-major so next-conv gathers start as
    soon as section 0 lands.
  - per 128-token chunk: dma_gather src rows and segment-sum on the PE:
    psum[feat, seg] += msg[tok, feat]^T @ S[tok, seg], S built on-chip
    (iota==segid).  Partial sums accumulate into zbuf (f32 SBUF) across
    sections; self loops folded in as a dinv^2 pre-scale of zbuf.
  - weight GEMM from fp16 staging after the last section, BN stats fused
    into eviction (accum_out), 1KB AllReduce, scalar affine+ReLU, table
    rebuild (PE transpose + dinv[src] scale + fp16 cast).
"""

import math
import os
import time
from contextlib import ExitStack

import numpy as np

CORES = 8
H = 128
EPS = 1e-5


class Cfg:
    def __init__(self, N, depth=9, sblk=8, cap=8, queues=1, scratch=16384):
        assert N % CORES == 0
        self.N = N
        self.S = N // CORES
        self.NPC = ((self.S + 127) // 128) * 128
        self.NBLK = self.NPC // 128
        self.depth = depth
        self.nconv = 2 * depth + 1
        self.SB = min(sblk, self.NBLK)     # dst blocks per superblock
        self.NSUP = (self.NBLK + self.SB - 1) // self.SB
        self.CAP = cap                     # max chunks per gather call
        self.GSB = 3                       # superblocks per psum group
        self.QUEUES = queues
        self.SCRATCH = scratch
        # table sections: pair-aligned slot ranges, one slice-AllGather each.
        # Packing runs at pair granularity (cap 1024) then splits each pair
        # into two balanced 128-windows.
        self.NPAIR = (self.NBLK + 1) // 2
        q, r = divmod(self.NPAIR, 4)
        secpairs = [q + (1 if i < r else 0) for i in range(4)]
        self.SECWINS = [2 * p for p in secpairs]
        self.SECWINS[3] -= 2 * sum(secpairs) - self.NBLK  # odd NBLK tail
        self.SECSTART = [0]
        for wdt in self.SECWINS:
            self.SECSTART.append(self.SECSTART[-1] + wdt)
        for wdt in self.SECWINS:
            assert 8 * wdt * 128 <= 32767  # gather idx fits int16
        self.SECPAIRS = secpairs
        # overflow pairs (may take 9 chunks = cap 1152), interleaved one per
        # section then a second; reserve pairs are repair-only spill
        ps = [0]
        for p in secpairs:
            ps.append(ps[-1] + p)
        ov1, ov2, ov3 = [], [], []
        self.RESPAIR = []
        for j in range(4):
            lo, span = ps[j], secpairs[j]
            ov1.append(lo + span // 6)
            ov2.append(lo + span // 2)
            ov3.append(lo + (5 * span) // 6)
            self.RESPAIR.append(lo + span // 3)
        self.OVPAIR = ov1 + ov2 + ov3
        self.PSECSTART = ps
        assert cap * 128 <= scratch // 16


FULL = Cfg(100000, queues=4, scratch=49152, cap=8)


# ----------------------------------------------------------------------------
# Host-side preprocessing (sharding / token planning)
# ----------------------------------------------------------------------------

def wrap16(a):
    # token i -> [i % 16, i // 16], replicated to 128 partitions
    b = a.astype(np.int16).reshape(-1, 16).T.copy()
    return np.tile(b, (8, 1))


def balance_nodes(src, dst, N, cfg):
    """Permute nodes -> (core, slot).  Pass 1 equalizes per-core in-edge
    totals; pass A seeds sections (slot ranges) balancing out-degree; pass B
    packs each core's nodes into 256-slot pairs so per-(core, src-section,
    pair) token counts stay <= 1024 (1152 at designated overflow pairs);
    pass C splits each pair into two balanced 128-windows."""
    NPAIR = cfg.NPAIR
    deg_in = np.bincount(dst, minlength=N).astype(np.int64)
    deg_out = np.bincount(src, minlength=N).astype(np.int64)
    sec_of_pair = np.repeat(np.arange(4), cfg.SECPAIRS)

    # ---- pass 1: nodes -> cores (snake over degree-sorted nodes) ----
    order = np.argsort(-deg_in, kind="stable")
    node_core = np.empty(N, np.int8)
    blk = order.reshape(-1, CORES)
    snake = np.arange(CORES)
    for i, row in enumerate(blk):
        node_core[row] = snake if i % 2 == 0 else snake[::-1]

    # ---- pass A: seed sections balancing per-section OUT-degree ----
    node_sec = np.empty(N, np.int8)
    per_core_nodes = []
    for k in range(CORES):
        nodes = np.where(node_core == k)[0]
        per_core_nodes.append(nodes[np.argsort(-deg_in[nodes],
                                               kind="stable")])
        byout = nodes[np.argsort(-deg_out[nodes], kind="stable")]
        seccap = np.array([w * 128 for w in cfg.SECWINS])
        # two LPT iterations: the second starts from the first's final
        # imbalance as a bias, cancelling the slot-capacity tail skew
        bias = np.zeros(4, np.int64)
        for it in range(2):
            fill = np.zeros(4, np.int64)
            tot = bias.copy()
            for n in byout:
                t = np.where(fill < seccap, tot, 1 << 60)
                j = int(t.argmin())
                node_sec[n] = j
                fill[j] += 1
                tot[j] += deg_out[n]
            bias = tot - bias - (tot - bias).mean().astype(np.int64)

    # ---- per-node in-edge profile by src section ----
    prof = np.zeros((N, 4), np.int32)
    np.add.at(prof, (dst, node_sec[src]), 1)

    # ---- pass B: per-core pair packing within sections ----
    node_slot = np.empty(N, np.int32)
    capp = np.full((NPAIR, 4), 1024, np.int64)
    capp[cfg.OVPAIR] = 1152
    capp[cfg.RESPAIR] = 1152
    secmask = np.zeros((4, NPAIR), bool)
    for j in range(4):
        secmask[j, cfg.PSECSTART[j]:cfg.PSECSTART[j + 1]] = True
    pslots = np.array([min(256, cfg.NBLK * 128 - 256 * p)
                       for p in range(NPAIR)])

    for k in range(CORES):
        nodes = per_core_nodes[k]
        v = prof[nodes]                               # [n, 4] degree-sorted
        nsec = node_sec[nodes]
        C = np.zeros((NPAIR, 4), np.int64)
        slots_left = pslots.copy()
        win_of = np.empty(len(nodes), np.int32)
        Tcol = np.maximum(v.sum(axis=0), 1)           # [4]
        weights = np.full((NPAIR, 4), 8.0)
        weights[cfg.OVPAIR] = 9.0
        t_wq = Tcol[None, :] * weights / weights.sum(axis=0)[None, :]
        for i in range(len(nodes)):
            after = C + v[i]
            score = (after / t_wq).max(axis=1) \
                + 1e6 * ((after > capp) & (C <= capp)).sum(axis=1) \
                + np.where(slots_left > 0, 0.0, 1e9) \
                + np.where(secmask[nsec[i]], 0.0, 1e9)
            w = int(score.argmin())
            win_of[i] = w
            C[w] += v[i]
            slots_left[w] -= 1
        # repair: move nodes out of over-cap pairs (same section only)
        stuck = set()
        for _ in range(3000):
            over = np.argwhere(C > capp)
            over = [t for t in over if (t[0], t[1]) not in stuck]
            if not over:
                break
            w1, q = over[0]
            in_w1 = np.where(win_of == w1)[0]
            cand = in_w1[v[in_w1, q] > 0]
            by_q = np.argsort(-v[cand, q], kind="stable")
            cand = np.concatenate([cand[by_q[:8]], cand[by_q[::-1][:8]]])
            sject = sec_of_pair[w1]
            moved = False
            for a in cand:
                room = capp - (C + v[a])              # [NPAIR, 4]
                ok = (room >= 0).all(axis=1) & (slots_left > 0) \
                    & secmask[sject]
                ok[w1] = False
                if ok.any():
                    w2 = int(np.argmax(np.where(ok, room[:, q], -1)))
                    win_of[a] = w2
                    C[w1] -= v[a]
                    C[w2] += v[a]
                    slots_left[w1] += 1
                    slots_left[w2] -= 1
                    moved = True
                    break
                # swap with a low-v[q] node from a same-section pair
                for w2 in np.argsort(np.where(secmask[sject],
                                              C[:, q] - capp[:, q],
                                              1 << 40))[:12]:
                    if w2 == w1 or not secmask[sject][w2]:
                        continue
                    in_w2 = np.where(win_of == w2)[0]
                    if len(in_w2) == 0:
                        continue
                    bs = in_w2[np.argsort(v[in_w2, q], kind="stable")[:4]]
                    for b in bs:
                        if v[b, q] >= v[a, q]:
                            break
                        nC1 = C[w1] - v[a] + v[b]
                        nC2 = C[w2] + v[a] - v[b]
                        if (nC2 > capp[w2]).any() or \
                                (nC1 > capp[w1]).sum() > \
                                (C[w1] > capp[w1]).sum():
                            continue
                        win_of[a], win_of[b] = w2, w1
                        C[w1], C[w2] = nC1, nC2
                        moved = True
                        break
                    if moved:
                        break
                if moved:
                    break
            if not moved:
                stuck.add((int(w1), int(q)))
        # ---- pass C: split each pair into two 128-windows, both under the
        # per-window cap (512, or 576 in overflow pairs) ----
        fill = np.zeros(2 * NPAIR, np.int64)
        for p in range(NPAIR):
            members = np.where(win_of == p)[0]
            if len(members) == 0:
                continue
            mv = v[members]
            o = np.argsort(-mv.sum(axis=1), kind="stable")
            members, mv = members[o], mv[o]
            half = np.zeros((2, 4), np.int64)
            hfill = np.zeros(2, np.int64)
            hcap = np.array([128, min(128, pslots[p] - 128)])
            wcap = capp[p] // 2                       # [4] per-window caps
            tpair = np.maximum(mv.sum(axis=0), 1)
            for i2, n2 in enumerate(members):
                aft = half + mv[i2]                   # [2, 4]
                sc = (aft / tpair).max(axis=1) \
                    + 1e6 * ((aft > wcap) & (half <= wcap)).sum(axis=1) \
                    + np.where(hfill < hcap, 0.0, 1e9)
                h = int(sc.argmin())
                wslot = 2 * p + h
                node_slot[nodes[n2]] = wslot * 128 + fill[wslot]
                fill[wslot] += 1
                half[h] += mv[i2]
                hfill[h] += 1
    return node_core.astype(np.int64), node_slot.astype(np.int64)


def preprocess(x, edge_index, cfg):
    N, S, NPC, NBLK, SB, CAP = (cfg.N, cfg.S, cfg.NPC, cfg.NBLK, cfg.SB,
                                cfg.CAP)
    # self loops are folded in as a dinv^2 pre-scale of zbuf (no tokens)
    src = np.asarray(edge_index[0], np.int64)
    dst = np.asarray(edge_index[1], np.int64)
    deg = np.bincount(dst, minlength=N).astype(np.float32) + 1.0
    dinv = (1.0 / np.sqrt(deg)).astype(np.float32)

    node_core, node_slot = balance_nodes(src, dst, N, cfg)

    NWIN = NBLK
    NG = 4 * NWIN                          # (section, window) groups
    sec_of_win = np.repeat(np.arange(4), cfg.SECWINS)
    secstart = np.asarray(cfg.SECSTART[:4])
    secrows = np.asarray([w * 128 for w in cfg.SECWINS])

    ssec = sec_of_win[node_slot[src] // 128]          # [E] src section
    lrow = (node_core[src] * secrows[ssec]
            + node_slot[src] - secstart[ssec] * 128)  # row in section table
    shard = node_core[dst]

    per_core = []
    cnt = np.zeros((CORES, NG), np.int64)
    for k in range(CORES):
        m = shard == k
        d = node_slot[dst[m]]
        key = ssec[m] * NWIN + d // 128
        order = np.argsort(key, kind="stable")
        per_core.append((lrow[m][order], (d % 128)[order]))
        bounds = np.searchsorted(key[order], np.arange(NG + 1))
        per_core[k] = per_core[k] + (bounds,)
        cnt[k] = np.diff(bounds)

    nch = ((cnt + 127) // 128).max(axis=0)             # [NG] static plan

    # Superblocks are processed in groups of GSB whose psum banks stay
    # resident across all 4 table sections (6 banks + 2 gemm = 8 total);
    # start/stop flags per (group, superblock, bank) span the sections.
    # plan[g][j] -> list of per-superblock nodes (sb, calls).
    GSB = cfg.GSB
    NGRP = (cfg.NSUP + GSB - 1) // GSB
    plan = []
    segcol = 0
    off16 = 0
    for g in range(NGRP):
        sbs = list(range(g * GSB, min((g + 1) * GSB, cfg.NSUP)))
        bank_n = {}
        for sbi in sbs:
            wins = list(range(sbi * SB, min((sbi + 1) * SB, NWIN)))
            for wb in wins:
                bk = (sbi, (wb - sbi * SB) // 4)
                bank_n[bk] = bank_n.get(bk, 0) + sum(
                    int(nch[j * NWIN + wb]) for j in range(4))
        bank_seen = {bk: 0 for bk in bank_n}
        gplan = []
        for j in range(4):
            jnodes = []
            for sbi in sbs:
                wins = list(range(sbi * SB, min((sbi + 1) * SB, NWIN)))
                chunk_ids = [(wb, i) for wb in wins
                             for i in range(int(nch[j * NWIN + wb]))]
                calls = []
                pos = 0
                while pos < len(chunk_ids):
                    take = chunk_ids[pos:pos + CAP]
                    # partial (highest-i) chunks last so their pads become
                    # a trailing run of negative idxs the ucode trims
                    take = sorted(take, key=lambda wi: wi[1])
                    descs = []
                    for jslot, (wb, i) in enumerate(take):
                        bk = (sbi, (wb - sbi * SB) // 4)
                        first = bank_seen[bk] == 0
                        bank_seen[bk] += 1
                        last = bank_seen[bk] == bank_n[bk]
                        descs.append((jslot, wb - sbi * SB, segcol,
                                      bool(first), bool(last)))
                        segcol += 1
                    calls.append(dict(n=len(take), off16=off16, descs=descs,
                                      chunks=take))
                    off16 += len(take) * 8
                    pos += len(take)
                jnodes.append(dict(sb=sbi, calls=calls))
            gplan.append(jnodes)
        plan.append(gplan)
    nchk = segcol

    in_maps = []
    for k in range(CORES):
        lrow_k, seg_k, bounds = per_core[k]
        idx_cols, seg_cols = [], []
        for gplan in plan:
          for j in range(4):
            for node in gplan[j]:
                for call in node["calls"]:
                    L, Sg = [], []
                    for ci, (wb, i) in enumerate(call["chunks"]):
                        gq = j * NWIN + wb
                        lo, hi = int(bounds[gq]), int(bounds[gq + 1])
                        s0 = lo + i * 128
                        last = ci == len(call["chunks"]) - 1
                        rows = np.zeros(128, np.int64)
                        segs = np.full(128, -1, np.int64)
                        n = max(0, min(hi - s0, 128))
                        if n > 0:
                            rows[:n] = lrow_k[s0:s0 + n]
                            segs[:n] = seg_k[s0:s0 + n]
                            if last:
                                rows[n:] = -1  # ucode trims trailing negs
                        L.append(rows)
                        Sg.append(segs)
                    idx_cols.append(wrap16(np.concatenate(L)))
                    seg_cols.append(np.stack(Sg))
        IDX = np.concatenate(idx_cols, axis=1)
        SEGID = np.ascontiguousarray(
            np.concatenate(seg_cols, axis=0).T.astype(np.float16))

        nodes_k = np.where(node_core == k)[0]
        slots_k = node_slot[nodes_k]
        xt = np.zeros((x.shape[1], NPC), dtype=np.float32)
        xt[:, slots_k] = np.asarray(x[nodes_k], np.float32).T
        dv = np.zeros(NPC, dtype=np.float32)
        dv[slots_k] = dinv[nodes_k]
        dinv_nm = np.ascontiguousarray(dv.reshape(NBLK, 128).T)
        dinvb = np.ascontiguousarray(
            np.broadcast_to(dv, (128, NPC)).astype(np.float16))
        in_maps.append({"xT": xt, "gidx": IDX, "segid": SEGID,
                        "dinv_nm": dinv_nm, "dinvb": dinvb})
    return in_maps, plan, nchk, (node_core, node_slot)


# ----------------------------------------------------------------------------
# Device kernel
# ----------------------------------------------------------------------------

def build_nc(cfg, plan, nchk, d_in):
    import concourse.bacc as bacc
    import concourse.bass as bass
    import concourse.mybir as mybir
    import concourse.tile as tile

    f32 = mybir.dt.float32
    f16 = mybir.dt.float16
    i16 = mybir.dt.int16
    AF = mybir.ActivationFunctionType
    ALU = mybir.AluOpType
    AX = mybir.AxisListType

    NPC, NBLK, SB, GSB = cfg.NPC, cfg.NBLK, cfg.SB, cfg.GSB
    depth = cfg.depth
    nconv = cfg.nconv
    MAXSLOT = max(c["n"] for gp in plan for jn in gp for nd in jn
                  for c in nd["calls"])
    NS = (NPC + 511) // 512
    STATC = 2 * cfg.NSUP + 2

    nc = bacc.Bacc("TRN2", target_bir_lowering=False, debug=False,
                   num_devices=CORES,
                   dynamic_dma_scratch_size=cfg.SCRATCH,
                   num_swdge_queues=cfg.QUEUES)

    # ---- I/O ----
    xT_d = nc.dram_tensor("xT", [d_in, NPC], f32, kind="ExternalInput")
    gidx_d = nc.dram_tensor("gidx", [128, nchk * 8], i16, kind="ExternalInput")
    segid_d = nc.dram_tensor("segid", [128, nchk], f16, kind="ExternalInput")
    dinvnm_d = nc.dram_tensor("dinv_nm", [128, NBLK], f32, kind="ExternalInput")
    dinvb_d = nc.dram_tensor("dinvb", [128, NPC], f16, kind="ExternalInput")
    W0_d = nc.dram_tensor("W0", [d_in, H], f32, kind="ExternalInput")
    Ws1_d = nc.dram_tensor("Ws1", [depth, H, H], f16, kind="ExternalInput")
    Ws2_d = nc.dram_tensor("Ws2", [depth - 1, H, H], f16, kind="ExternalInput")
    Wout_d = nc.dram_tensor("Wout", [H, 1], f16, kind="ExternalInput")
    g1_d = nc.dram_tensor("g1T", [H, depth + 1], f32, kind="ExternalInput")
    b1_d = nc.dram_tensor("b1T", [H, depth + 1], f32, kind="ExternalInput")
    g2_d = nc.dram_tensor("g2T", [H, depth - 1], f32, kind="ExternalInput")
    b2_d = nc.dram_tensor("b2T", [H, depth - 1], f32, kind="ExternalInput")
    ident_d = nc.dram_tensor("ident", [128, 128], f32, kind="ExternalInput")
    out_d = nc.dram_tensor("out", [1, NPC], f32, kind="ExternalOutput")

    # ---- internals ----
    tabs = [[nc.dram_tensor(f"tab{i}_{j}",
                            [CORES * cfg.SECWINS[j] * 128, H], f16,
                            addr_space="Shared")
             for j in range(4)] for i in range(2)]
    ulocal = nc.dram_tensor("ulocal", [NPC, H], f16)
    stats_in = nc.dram_tensor("stats_in", [128, 2], f32)
    stats_out = nc.dram_tensor("stats_out", [128, 2], f32, addr_space="Shared")
    xs_d = nc.dram_tensor("xs", [depth, 128, NPC], f32)

    rg = [list(range(CORES))]
    SECROW = [s * 128 for s in cfg.SECSTART]

    with tile.TileContext(nc, num_cores=CORES) as tc, ExitStack() as ctx:
        persist = ctx.enter_context(tc.tile_pool(name="persist", bufs=1))
        msgp = ctx.enter_context(tc.tile_pool(name="msg", bufs=8))
        sp = ctx.enter_context(tc.tile_pool(name="sbld", bufs=5))
        ytp = ctx.enter_context(tc.tile_pool(name="yt", bufs=3))
        stgp = ctx.enter_context(tc.tile_pool(name="stg", bufs=3))
        wp = ctx.enter_context(tc.tile_pool(name="wp", bufs=2))
        skp = ctx.enter_context(tc.tile_pool(name="skp", bufs=3))
        smallp = ctx.enter_context(tc.tile_pool(name="small", bufs=8))
        obp = ctx.enter_context(tc.tile_pool(name="obp", bufs=2))
        accp = ctx.enter_context(tc.tile_pool(name="accp", bufs=1, space="PSUM"))
        pgemm = ctx.enter_context(tc.tile_pool(name="pgemm", bufs=2, space="PSUM"))

        # persistent tiles
        zbuf = persist.tile([128, NPC], f32)
        idx_sb = persist.tile([128, nchk * 8], i16)
        segid_sb = persist.tile([128, nchk], f16)
        dinvb_sb = persist.tile([128, NPC], f16)
        dinvnm_sb = persist.tile([128, NBLK], f32)
        iota_sb = persist.tile([128, 128], f16)
        ident_sb = persist.tile([128, 128], f32)
        sums_sb = persist.tile([128, STATC], f32)
        sumsq_sb = persist.tile([128, STATC], f32)
        stat2_sb = persist.tile([128, 2], f32)
        sqscr = persist.tile([128, 512], f32)
        wout_sb = persist.tile([128, 1], f16)

        # load persistent data (split large loads across DMA queues)
        PIECE = 8192 * 2  # int16 elems per partition-row piece
        tot16 = nchk * 8
        o = 0
        while o < tot16:
            w = min(PIECE, tot16 - o)
            nc.sync.dma_start(out=idx_sb[:, o:o + w], in_=gidx_d[:, o:o + w])
            o += w
        nc.sync.dma_start(out=segid_sb[:], in_=segid_d[:])
        o = 0
        while o < NPC:
            w = min(4096, NPC - o)
            nc.sync.dma_start(out=dinvb_sb[:, o:o + w], in_=dinvb_d[:, o:o + w])
            o += w
        nc.sync.dma_start(out=dinvnm_sb[:], in_=dinvnm_d[:])
        nc.sync.dma_start(out=ident_sb[:], in_=ident_d[:])
        nc.sync.dma_start(out=wout_sb[:], in_=Wout_d[:])
        nc.gpsimd.iota(iota_sb[:], pattern=[[1, 128]], base=0,
                       channel_multiplier=0,
                       allow_small_or_imprecise_dtypes=True)
        # zero-init msg pool buffers: gather pad-trimming leaves slots
        # unwritten, and 0 * NaN garbage would poison the PE segment-sums
        for _ in range(8):
            mz = msgp.tile([128, MAXSLOT, H], f16, tag="msg")
            nc.vector.memset(mz[:], 0.0)

        def gemm_weight(t):
            if t == 1 or t == nconv:
                return None
            w = wp.tile([128, 128], f16, tag="w")
            if t <= depth + 1:
                nc.sync.dma_start(out=w[:], in_=Ws1_d[t - 2])
            else:
                nc.sync.dma_start(out=w[:], in_=Ws2_d[t - depth - 2])
            return w

        def bn_params(t):
            gt = smallp.tile([128, 1], f32, tag="gt")
            bt = smallp.tile([128, 1], f32, tag="bt")
            if t <= depth + 1:
                nc.sync.dma_start(out=gt[:], in_=g1_d[:, t - 1:t])
                nc.sync.dma_start(out=bt[:], in_=b1_d[:, t - 1:t])
            else:
                i = t - depth - 2
                nc.sync.dma_start(out=gt[:], in_=g2_d[:, i:i + 1])
                nc.sync.dma_start(out=bt[:], in_=b2_d[:, i:i + 1])
            return gt, bt

        def build_table(t):
            # zbuf (feature-major fp32) -> transpose -> dinv[src] -> fp16;
            # each section's slice-AllGather fires as soon as its rows are
            # staged so distribution overlaps the rest of the rebuild.
            NB4 = (NBLK + 3) // 4
            agpt = {}
            for j in range(4):
                agpt[(SECROW[j + 1] + 511) // 512 - 1] = j
            for g in range(NB4):
                b0 = 4 * g
                nb = min(4, NBLK - b0)
                st = stgp.tile([128, 4, H], f16, tag="st")
                pt = pgemm.tile([128, 512], f32, tag="pg", name="pt")
                for b_ in range(nb):
                    b = b0 + b_
                    nc.tensor.transpose(
                        pt[:, b_ * 128:(b_ + 1) * 128],
                        zbuf[:, b * 128:(b + 1) * 128], ident_sb[:])
                    nc.vector.tensor_scalar_mul(
                        st[:, b_, :], pt[:, b_ * 128:(b_ + 1) * 128],
                        dinvnm_sb[:, b:b + 1])
                nc.sync.dma_start(
                    out=ulocal[b0 * 128:(b0 + nb) * 128, :]
                    .rearrange("(a p) f -> p a f", p=128),
                    in_=st[:, :nb, :])
                if g in agpt:
                    j = agpt[g]
                    nc.gpsimd.collective_compute(
                        "AllGather", ALU.bypass, replica_groups=rg,
                        ins=[ulocal[SECROW[j]:SECROW[j + 1], :]],
                        outs=[tabs[t % 2][j][:, :]])

        # ---- stage 0: z0.T = W0.T @ xT ----
        w0 = persist.tile([d_in, H], f32)
        nc.sync.dma_start(out=w0[:], in_=W0_d[:])
        for s in range(NS):
            c0 = s * 512
            cw = min(512, NPC - c0)
            xt = skp.tile([d_in, 512], f32, tag="xt")
            nc.sync.dma_start(out=xt[:, :cw], in_=xT_d[:, c0:c0 + cw])
            pg = pgemm.tile([128, 512], f32, tag="pg")
            nc.tensor.matmul(pg[:, :cw], w0[:], xt[:, :cw],
                             start=True, stop=True)
            nc.scalar.copy(zbuf[:, c0:c0 + cw], pg[:, :cw])
        build_table(0)

        # ---- conv layers ----
        qrr = 0
        for t in range(1, nconv + 1):
            w = gemm_weight(t)
            scol = 0
            for gplan in plan:
                # psum banks for this superblock group stay resident across
                # all 4 table sections
                acct = {}
                for node in gplan[0]:
                    sbi = node["sb"]
                    nb_sb = min(SB, NBLK - sbi * SB)
                    for b in range((nb_sb + 3) // 4):
                        acct[(sbi, b)] = accp.tile(
                            [128, 512], f32, tag=f"acct{sbi % GSB}_{b}",
                            name=f"acct{sbi % GSB}_{b}")
                for j in range(4):
                    tabj = tabs[(t - 1) % 2][j]
                    for node in gplan[j]:
                        sbi = node["sb"]
                        for call in node["calls"]:
                            ncall, off16 = call["n"], call["off16"]
                            msg = msgp.tile([128, MAXSLOT, H], f16, tag="msg")
                            nc.gpsimd.dma_gather(
                                msg[:, :ncall, :], tabj[:, :],
                                idx_sb[:, off16:off16 + ncall * 8],
                                ncall * 128, ncall * 128, H,
                                queue_num=qrr % cfg.QUEUES)
                            qrr += 1
                            c0 = call["descs"][0][2]
                            st_ = sp.tile([128, MAXSLOT, 128], f16, tag="S")
                            nc.vector.tensor_tensor(
                                st_[:, :ncall, :],
                                iota_sb[:].unsqueeze(1)
                                .broadcast_to([128, ncall, 128]),
                                segid_sb[:, c0:c0 + ncall].unsqueeze(2)
                                .broadcast_to([128, ncall, 128]),
                                op=ALU.is_equal)
                            for (jslot, jp, segc, first, last) in call["descs"]:
                                nc.tensor.matmul(
                                    acct[(sbi, jp // 4)][:, (jp % 4) * 128:
                                                         (jp % 4) * 128 + 128],
                                    msg[:, jslot, :], st_[:, jslot, :],
                                    start=first, stop=last)

                # ---- evict group (y*dinv + self loop dinv^2*z_prev) ----
                for node in gplan[0]:
                    sbi = node["sb"]
                    nb_sb = min(SB, NBLK - sbi * SB)
                    nb0 = sbi * SB * 128
                    accs = [acct[(sbi, jb // 4)][:, (jb % 4) * 128:
                                                 (jb % 4 + 1) * 128]
                            for jb in range(nb_sb)]
                    if t == 1:
                        for jb in range(nb_sb):
                            cols = slice(nb0 + jb * 128, nb0 + (jb + 1) * 128)
                            stmp = ytp.tile([128, 128], f16, tag="slf",
                                            name="stmp")
                            nc.vector.tensor_mul(stmp[:], zbuf[:, cols],
                                                 dinvb_sb[:, cols])
                            nc.vector.tensor_mul(stmp[:], stmp[:],
                                                 dinvb_sb[:, cols])
                            nc.vector.tensor_mul(zbuf[:, cols], accs[jb],
                                                 dinvb_sb[:, cols])
                            nc.vector.tensor_add(zbuf[:, cols], zbuf[:, cols],
                                                 stmp[:])
                        continue
                    ytmp = ytp.tile([128, SB * 128], f16, tag="ytmp")
                    for jb in range(nb_sb):
                        cols = slice(nb0 + jb * 128, nb0 + (jb + 1) * 128)
                        ycols = slice(jb * 128, (jb + 1) * 128)
                        stmp = ytp.tile([128, 128], f16, tag="slf",
                                        name="stmp")
                        nc.vector.tensor_mul(stmp[:], zbuf[:, cols],
                                             dinvb_sb[:, cols])
                        nc.vector.tensor_mul(stmp[:], stmp[:],
                                             dinvb_sb[:, cols])
                        nc.vector.tensor_mul(ytmp[:, ycols], accs[jb],
                                             dinvb_sb[:, cols])
                        nc.vector.tensor_add(ytmp[:, ycols], ytmp[:, ycols],
                                             stmp[:])
                    for hw_ in range(0, nb_sb * 128, 512):
                        cw = min(512, nb_sb * 128 - hw_)
                        cols = slice(nb0 + hw_, nb0 + hw_ + cw)
                        if t < nconv:
                            pg = pgemm.tile([128, 512], f32, tag="pg")
                            nc.tensor.matmul(pg[:, :cw], w[:],
                                             ytmp[:, hw_:hw_ + cw],
                                             start=True, stop=True)
                            nc.scalar.activation(
                                zbuf[:, cols], pg[:, :cw],
                                AF.Copy, accum_out=sums_sb[:, scol:scol + 1])
                            nc.scalar.activation(
                                sqscr[:, :cw], pg[:, :cw],
                                AF.Square,
                                accum_out=sumsq_sb[:, scol:scol + 1])
                            scol += 1
                        else:
                            po = pgemm.tile([128, 512], f32, tag="pg",
                                            name="po")
                            nc.tensor.matmul(po[0:1, :cw], wout_sb[:],
                                             ytmp[:, hw_:hw_ + cw],
                                             start=True, stop=True)
                            ob = obp.tile([1, 512], f32, tag="ob")
                            nc.scalar.activation(ob[:, :cw], po[0:1, :cw],
                                                 AF.Sigmoid)
                            nc.sync.dma_start(
                                out=out_d[:, nb0 + hw_: nb0 + hw_ + cw],
                                in_=ob[:, :cw])

            if t == nconv:
                break

            # ---- BN stats ----
            if t == 1:
                for s in range(NS):
                    c0 = s * 512
                    cw = min(512, NPC - c0)
                    zsl = zbuf[:, c0:c0 + cw]
                    nc.vector.tensor_reduce(sums_sb[:, s:s + 1], zsl,
                                            axis=AX.X, op=ALU.add)
                    nc.vector.tensor_mul(sqscr[:, :cw], zsl, zsl)
                    nc.vector.tensor_reduce(sumsq_sb[:, s:s + 1], sqscr[:, :cw],
                                            axis=AX.X, op=ALU.add)
                scol = NS
            nc.vector.tensor_reduce(stat2_sb[:, 0:1], sums_sb[:, :scol],
                                    axis=AX.X, op=ALU.add)
            nc.vector.tensor_reduce(stat2_sb[:, 1:2], sumsq_sb[:, :scol],
                                    axis=AX.X, op=ALU.add)
            nc.sync.dma_start(out=stats_in[:, :], in_=stat2_sb[:])
            nc.gpsimd.collective_compute(
                "AllReduce", ALU.add, replica_groups=rg,
                ins=[stats_in[:, :]], outs=[stats_out[:, :]])
            gst = smallp.tile([128, 2], f32, tag="gst")
            nc.sync.dma_start(out=gst[:], in_=stats_out[:, :])

            mean = smallp.tile([128, 1], f32, tag="mean")
            m2 = smallp.tile([128, 1], f32, tag="m2")
            var = smallp.tile([128, 1], f32, tag="var")
            scl = smallp.tile([128, 1], f32, tag="scl")
            sft = smallp.tile([128, 1], f32, tag="sft")
            inv_n = 1.0 / float(cfg.N)
            nc.vector.tensor_scalar_mul(mean[:], gst[:, 0:1], inv_n)
            nc.vector.tensor_scalar_mul(var[:], gst[:, 1:2], inv_n)
            nc.vector.tensor_mul(m2[:], mean[:], mean[:])
            nc.vector.scalar_tensor_tensor(
                var[:], m2[:], -1.0, var[:], op0=ALU.mult, op1=ALU.add)
            nc.vector.tensor_scalar_add(var[:], var[:], EPS)
            gt, bt = bn_params(t)
            nc.scalar.sqrt(scl[:], var[:])
            nc.vector.reciprocal(scl[:], scl[:])
            nc.vector.tensor_mul(scl[:], scl[:], gt[:])
            nc.vector.tensor_mul(sft[:], mean[:], scl[:])
            nc.vector.scalar_tensor_tensor(
                sft[:], sft[:], -1.0, bt[:], op0=ALU.mult, op1=ALU.add)

            # ---- normalize + relu (in place on zbuf) ----
            for s in range(NS):
                c0 = s * 512
                cw = min(512, NPC - c0)
                nc.scalar.activation(zbuf[:, c0:c0 + cw], zbuf[:, c0:c0 + cw],
                                     AF.Relu, bias=sft[:], scale=scl[:])

            # ---- stash xs / skip add ----
            if t <= depth:
                for s in range(NS):
                    c0 = s * 512
                    cw = min(512, NPC - c0)
                    nc.sync.dma_start(out=xs_d[t - 1][:, c0:c0 + cw],
                                      in_=zbuf[:, c0:c0 + cw])
            if t + 1 >= depth + 2:
                jj = 2 * depth - t
                for s in range(NS):
                    c0 = s * 512
                    cw = min(512, NPC - c0)
                    sk = skp.tile([128, 512], f32, tag="sk")
                    nc.sync.dma_start(out=sk[:, :cw],
                                      in_=xs_d[jj][:, c0:c0 + cw])
                    nc.vector.tensor_add(zbuf[:, c0:c0 + cw],
                                         zbuf[:, c0:c0 + cw], sk[:, :cw])

            build_table(t)

    nc.compile()
    return nc


# ----------------------------------------------------------------------------
# Entry point
# ----------------------------------------------------------------------------

LAST_INFO = {}


def _run(inputs, cfg):
    from concourse.bass_utils import run_bass_kernel_spmd

    x = np.asarray(inputs["x"], np.float32)
    d_in = x.shape[1]
    t0 = time.time()
    shard_maps, plan, nchk, (node_core, node_slot) = preprocess(
        x, inputs["edge_index"], cfg)
    t1 = time.time()
    nc = build_nc(cfg, plan, nchk, d_in)
    t2 = time.time()

    common = {
        "W0": np.asarray(inputs["W0"], np.float32),
        "Ws1": np.asarray(inputs["Ws1"], np.float16),
        "Ws2": np.asarray(inputs["Ws2"], np.float16),
        "Wout": np.asarray(inputs["Wout"], np.float16),
        "g1T": np.ascontiguousarray(np.asarray(inputs["g1"], np.float32).T),
        "b1T": np.ascontiguousarray(np.asarray(inputs["b1"], np.float32).T),
        "g2T": np.ascontiguousarray(np.asarray(inputs["g2"], np.float32).T),
        "b2T": np.ascontiguousarray(np.asarray(inputs["b2"], np.float32).T),
        "ident": np.eye(128, dtype=np.float32),
    }
    in_maps = [dict(m, **common) for m in shard_maps]
    kw = {}
    if os.environ.get("KBENCH_TRACE"):
        kw = dict(trace=True, tmpdir=os.environ.get("KBENCH_TMPDIR") or None)
    res = run_bass_kernel_spmd(nc, in_maps, list(range(CORES)), **kw)
    t3 = time.time()
    LAST_INFO.update(preprocess_s=t1 - t0, build_s=t2 - t1, run_s=t3 - t2,
                     exec_time_ns=res.exec_time_ns, nchk=nchk)
    per_core = [res.results[k]["out"][0] for k in range(CORES)]
    out = np.empty(cfg.N, np.float32)
    for k in range(CORES):
        m = node_core == k
        out[m] = per_core[k][node_slot[m]]
    return out.reshape(cfg.N, 1).astype(np.float32)


def kernel(**inputs):
    return _run(inputs, FULL)
